# revision 18
# baseline (speedup 1.0000x reference)
"""Trainium2 Bass kernel for the GNN message-passing net (nn_Net_76690936037575).

Math: with assoc_var == arange(n_var) and assoc_con == arange(n_con) (the
spec-guaranteed fill), the scatter/scatter/gather pipeline collapses to

    out[0:n_con]      = head(con_mlp(con_node_features))
    out[n_con:n_var]  = head(var_mlp(var_node_features[n_con:n_var]))

where _mlp2 has no nonlinearity after its 2nd matmul, so that matmul fuses
with the head's 1st on the host:  M_s = sW2 @ W1, bM_s = sb2 @ W1 + b1.

Device dataflow (per core), activations laid out [128 dims x rows]:
  - 5 matmuls per 512-row tile in fp16 (K=2 input layer; 3x 128x128; the
    128->1 output layer col-tiled per PSUM bank via tile_position, with W4
    replicated to 32 cols so the whole bank is written).
  - bias+ReLU fused into ScalarE activation / VectorE tensor_scalar ops,
    one instruction per GROUP of 3 tiles over a 3-bank PSUM tile.
  - one Sigmoid per group; strided-partition DMA writes the output.

Sharding: rows split evenly across 8 cores (con 50k/core, var 25k/core),
weights replicated.
"""

import math

import numpy as np

DIM = 128
TILE_N = 512  # rows per matmul tile (one PSUM bank of f32)
GROUP = 4  # tiles per PSUM supertile / sigmoid group
N_CORES = 8

_NC_CACHE = {}


def _build_nc(ncon, nvar):
    """Build the per-core Bass program. ncon/nvar: rows per core."""
    import concourse.mybir as mybir
    import concourse.tile as tile
    from concourse import bacc

    dt = mybir.dt
    f32 = dt.float32
    f16 = dt.float16
    AF = mybir.ActivationFunctionType
    ALU = mybir.AluOpType

    nc = bacc.Bacc()

    GT = GROUP * TILE_N

    cfeat = nc.dram_tensor("cfeat", [2, ncon], f16, kind="ExternalInput")
    vfeat = nc.dram_tensor("vfeat", [2, nvar], f16, kind="ExternalInput")
    # all weights packed in one fp16 blob, all biases in one f32 blob, so the
    # constant preamble is 2 DMAs instead of 15 (keeps the warmup gap short)
    blob16 = nc.dram_tensor("blob16", [DIM, 800], f16, kind="ExternalInput")
    blob32 = nc.dram_tensor("blob32", [DIM, 7], f32, kind="ExternalInput")
    out_con = nc.dram_tensor("out_con", [ncon], f32, kind="ExternalOutput")
    out_var = nc.dram_tensor("out_var", [nvar], f32, kind="ExternalOutput")

    with tile.TileContext(nc) as tc:
        with (
            tc.tile_pool(name="const", bufs=1) as cpool,
            tc.tile_pool(name="feat", bufs=3) as fpool,
            tc.tile_pool(name="acts", bufs=10) as apool,
            tc.tile_pool(name="sig", bufs=2) as spool,
            tc.tile_pool(name="mm", bufs=2, space="PSUM") as mmpool,
        ):

            b16 = cpool.tile([DIM, 800], f16, tag="b16")
            nc.sync.dma_start(b16[:, :], blob16[:, :])
            b32 = cpool.tile([DIM, 7], f32, tag="b32")
            nc.sync.dma_start(b32[:, :], blob32[:, :])

            wmc_t = b16[:, 256:384]
            wmv_t = b16[:, 384:512]
            ww2_t = b16[:, 512:640]
            ww3_t = b16[:, 640:768]
            ww4_t = b16[:, 768:800]
            bb1c_t = b32[:, 0:1]
            bb1v_t = b32[:, 1:2]
            bbmc_t = b32[:, 2:3]
            bbmv_t = b32[:, 3:4]
            bb2_t = b32[:, 4:5]
            bb3_t = b32[:, 5:6]
            bb4_t = b32[:, 6:7]

            def halves(g_rows):
                if g_rows <= TILE_N:
                    return [(0, g_rows)]
                h = (g_rows // 2 + TILE_N - 1) // TILE_N * TILE_N
                h = min(h, g_rows)
                return [(0, h), (h, g_rows)] if h < g_rows else [(0, g_rows)]

            def mm_layer(w_t, src_t, njs):
                """One 128-K layer over a group: len(njs) matmuls into one
                multi-bank PSUM tile."""
                p = mmpool.tile([DIM, GT], f32, tag="mm")
                for j, nj in enumerate(njs):
                    nc.tensor.matmul(
                        p[:, j * TILE_N : j * TILE_N + nj],
                        w_t[:, :],
                        src_t[:, j * TILE_N : j * TILE_N + nj],
                        start=True,
                        stop=True,
                    )
                return p

            def emit_group_layers(metas):
                """Emit 1-2 groups in lockstep: PE streams group B's matmuls
                while group A's ReLU drains, keeping PE dense."""
                fts, p1s, t1s, t2s, t3s, t4s, p5s = {}, {}, {}, {}, {}, {}, {}
                for gi, m in metas.items():
                    ft = fpool.tile([DIM, TILE_N], f16, tag="feat")
                    for j, nj in enumerate(m["njs"]):
                        off = m["g0"] + j * TILE_N
                        nc.sync.dma_start(
                            ft[32 * j : 32 * j + 2, :nj],
                            m["feat"][:, off : off + nj],
                        )
                    fts[gi] = ft
                # layer 1: h = relu(x @ A1 + a1), K=2, the group's tiles
                # row-packed into concurrent PE row-groups via tile_position
                for gi, m in metas.items():
                    p1 = mmpool.tile([DIM, GT], f32, tag="mm")
                    for j, nj in enumerate(m["njs"]):
                        nc.tensor.matmul(
                            p1[:, j * TILE_N : j * TILE_N + nj],
                            b16[32 * j : 32 * j + 2, m["wa1c0"] : m["wa1c0"] + DIM],
                            fts[gi][32 * j : 32 * j + 2, :nj],
                            start=True,
                            stop=True,
                            tile_position=(32 * j, 0),
                        )
                    p1s[gi] = p1
                for gi, m in metas.items():
                    t1 = apool.tile([DIM, GT], f16, tag="acts")
                    for a, b in halves(m["g_rows"]):
                        nc.scalar.activation(
                            t1[:, a:b], p1s[gi][:, a:b],
                            AF.Relu, bias=m["b1"][:, :],
                        )
                    t1s[gi] = t1
                # layer 2 (fused mlp2 + head1): u = relu(h @ M + bM)
                for gi, m in metas.items():
                    p2 = mm_layer(m["wm"], t1s[gi], m["njs"])
                    t2 = apool.tile([DIM, GT], f16, tag="acts")
                    for a, b in halves(m["g_rows"]):
                        nc.vector.tensor_scalar(
                            t2[:, a:b], p2[:, a:b],
                            m["bm"][:, :], 0.0, ALU.add, ALU.max,
                        )
                    t2s[gi] = t2
                # layer 3: v = relu(u @ W2 + b2)
                for gi, m in metas.items():
                    p3 = mm_layer(ww2_t, t2s[gi], m["njs"])
                    t3 = apool.tile([DIM, GT], f16, tag="acts")
                    for a, b in halves(m["g_rows"]):
                        nc.scalar.activation(
                            t3[:, a:b], p3[:, a:b],
                            AF.Relu, bias=bb2_t[:, :],
                        )
                    t3s[gi] = t3
                # layer 4: w = relu(v @ W3 + b3)
                p4s = {}
                for gi, m in metas.items():
                    p4 = mm_layer(ww3_t, t3s[gi], m["njs"])
                    t4 = apool.tile([DIM, GT], f16, tag="acts")
                    for a, b in halves(m["g_rows"]):
                        nc.vector.tensor_scalar(
                            t4[:, a:b], p4[:, a:b],
                            bb3_t[:, :], 0.0, ALU.add, ALU.max,
                        )
                    t4s[gi] = t4
                    p4s[gi] = p4
                # layer 5: y = sigmoid(w @ W4 + b4), col-tiled; output goes
                # into p4's first bank, which ReLU4 has just drained
                for gi, m in metas.items():
                    p5 = p4s[gi]
                    for j, nj in enumerate(m["njs"]):
                        nc.tensor.matmul(
                            p5[32 * j : 32 * j + 32, :nj],
                            ww4_t[:, :],
                            t4s[gi][:, j * TILE_N : j * TILE_N + nj],
                            start=True,
                            stop=True,
                            tile_position=(0, 32 * j),
                        )
                    p5s[gi] = p5
                for gi, m in metas.items():
                    p5 = p5s[gi]
                    sg = spool.tile([DIM, TILE_N], f32, tag="sig")
                    if m["full"]:
                        k = len(m["njs"])
                        pcov = 32 * k
                        pmax = 32 * (k - 1) + 1
                        nc.scalar.activation(
                            sg[:pcov, :], p5[:pcov, 0:TILE_N], AF.Sigmoid,
                            bias=bb4_t[:pcov, :],
                        )
                        nc.sync.dma_start(
                            m["out"][m["g0"] : m["g0"] + m["g_rows"]].rearrange(
                                "(a b) -> a b", b=TILE_N
                            ),
                            sg[0:pmax:32, :],
                        )
                    else:
                        for j, t in enumerate(m["tiles"]):
                            nj = m["njs"][j]
                            nc.scalar.activation(
                                sg[32 * j : 32 * j + 1, :nj],
                                p5[32 * j : 32 * j + 1, :nj],
                                AF.Sigmoid,
                                bias=bb4_t[32 * j : 32 * j + 1, :],
                            )
                            nc.sync.dma_start(
                                m["out"][t * TILE_N : t * TILE_N + nj].rearrange(
                                    "(a b) -> a b", a=1
                                ),
                                sg[32 * j : 32 * j + 1, :nj],
                            )

            def stream_groups(feat, n_rows, wa1c0, b1_t, wm_t, bm_t, out):
                n_tiles = math.ceil(n_rows / TILE_N)
                n_groups = math.ceil(n_tiles / GROUP)
                metas = []
                for gi in range(n_groups):
                    tiles = list(range(gi * GROUP, min((gi + 1) * GROUP, n_tiles)))
                    njs = [min(TILE_N, n_rows - t * TILE_N) for t in tiles]
                    metas.append({
                        "feat": feat,
                        "out": out,
                        "wa1c0": wa1c0,
                        "b1": b1_t,
                        "wm": wm_t,
                        "bm": bm_t,
                        "tiles": tiles,
                        "njs": njs,
                        "g0": gi * GT,
                        "g_rows": min(GT, n_rows - gi * GT),
                        "full": all(n == TILE_N for n in njs),
                    })
                return metas

            # HAM warmup: ~5us of dense back-to-back matmuls flips the PE
            # clock gate from 1.2 to 2.4 GHz before the main stream begins.
            wp = mmpool.tile([DIM, GT], f32, tag="mm")
            for _ in range(14):
                nc.tensor.matmul(
                    wp[:, :TILE_N], b16[:, 0:128], b16[:, 288:800],
                    start=True, stop=True,
                )

            con_groups = stream_groups(
                cfeat, ncon, 0, bb1c_t, wmc_t, bbmc_t, out_con
            )
            var_groups = stream_groups(
                vfeat, nvar, 128, bb1v_t, wmv_t, bbmv_t, out_var
            )
            # interleave the two independent streams so one group's matmuls
            # overlap the other's ReLU drains
            order = []
            i = j = 0
            while i < len(con_groups) or j < len(var_groups):
                if i < len(con_groups):
                    order.append(con_groups[i]); i += 1
                if j < len(var_groups):
                    order.append(var_groups[j]); j += 1
            k = 0
            while k < len(order):
                pair = order[k : k + 2]
                emit_group_layers({n: m for n, m in enumerate(pair)})
                k += len(pair)

    nc.compile()
    return nc


def _make_in_maps(inputs, ncon_per, nvar_per):
    """Host-side sharding: transpose features, split rows, fuse weights."""
    f32 = np.float32
    f16 = np.float16
    cf = np.asarray(inputs["con_node_features"], f32)
    vf = np.asarray(inputs["var_node_features"], f32)
    n_con = cf.shape[0]
    n_var = vf.shape[0]

    W1 = np.asarray(inputs["W1"], f32)
    b1 = np.asarray(inputs["b1"], f32)
    mc = np.asarray(inputs["cW2"], f32) @ W1
    bmc = np.asarray(inputs["cb2"], f32) @ W1 + b1
    mv = np.asarray(inputs["vW2"], f32) @ W1
    bmv = np.asarray(inputs["vb2"], f32) @ W1 + b1

    conT = np.zeros((2, ncon_per * N_CORES), f16)
    conT[:, :n_con] = cf.T
    varT = np.zeros((2, nvar_per * N_CORES), f16)
    varT[:, : n_var - n_con] = vf[n_con:].T

    blob16 = np.zeros((DIM, 800), f16)
    for j in range(4):
        blob16[32 * j : 32 * j + 2, 0:128] = np.asarray(
            inputs["cW1"], f32
        ).astype(f16)
        blob16[32 * j : 32 * j + 2, 128:256] = np.asarray(
            inputs["vW1"], f32
        ).astype(f16)
    blob16[:, 256:384] = mc.astype(f16)
    blob16[:, 384:512] = mv.astype(f16)
    blob16[:, 512:640] = np.asarray(inputs["W2"], f32).astype(f16)
    blob16[:, 640:768] = np.asarray(inputs["W3"], f32).astype(f16)
    blob16[:, 768:800] = np.repeat(
        np.asarray(inputs["W4"], f32).reshape(DIM, 1), 32, axis=1
    ).astype(f16)
    blob32 = np.zeros((DIM, 7), f32)
    blob32[:, 0] = np.asarray(inputs["cb1"], f32)
    blob32[:, 1] = np.asarray(inputs["vb1"], f32)
    blob32[:, 2] = bmc
    blob32[:, 3] = bmv
    blob32[:, 4] = np.asarray(inputs["b2"], f32)
    blob32[:, 5] = np.asarray(inputs["b3"], f32)
    blob32[:, 6] = np.asarray(inputs["b4"], f32).reshape(-1)[0]

    shared = {"blob16": blob16, "blob32": blob32}
    in_maps = []
    for i in range(N_CORES):
        m = dict(shared)
        m["cfeat"] = np.ascontiguousarray(conT[:, i * ncon_per : (i + 1) * ncon_per])
        m["vfeat"] = np.ascontiguousarray(varT[:, i * nvar_per : (i + 1) * nvar_per])
        in_maps.append(m)
    return in_maps


def _reference_numpy(inputs):
    """General fallback (non-arange assoc indices): plain numpy."""
    f32 = np.float32

    def mlp2(x, W1, b1, W2, b2):
        return np.maximum(x @ W1 + b1, 0.0) @ W2 + b2

    vf = np.asarray(inputs["var_node_features"], f32)
    cf = np.asarray(inputs["con_node_features"], f32)
    av = np.asarray(inputs["assoc_var"])
    ac = np.asarray(inputs["assoc_con"])
    n = mlp2(vf, inputs["vW1"], inputs["vb1"], inputs["vW2"], inputs["vb2"])
    e = mlp2(cf, inputs["cW1"], inputs["cb1"], inputs["cW2"], inputs["cb2"])
    x = np.zeros((np.asarray(inputs["node_types"]).shape[0], n.shape[-1]), f32)
    x[av] = n
    x[ac] = e
    x = x[av]
    x = np.maximum(x @ inputs["W1"] + inputs["b1"], 0.0)
    x = np.maximum(x @ inputs["W2"] + inputs["b2"], 0.0)
    x = np.maximum(x @ inputs["W3"] + inputs["b3"], 0.0)
    x = x @ inputs["W4"] + inputs["b4"]
    return (1.0 / (1.0 + np.exp(-x))).astype(f32).squeeze(-1)


def kernel(**inputs):
    from concourse.bass_utils import run_bass_kernel_spmd

    cf = np.asarray(inputs["con_node_features"])
    vf = np.asarray(inputs["var_node_features"])
    av = np.asarray(inputs["assoc_var"])
    ac = np.asarray(inputs["assoc_con"])
    n_con = cf.shape[0]
    n_var = vf.shape[0]

    fast = (
        n_con <= n_var
        and av.shape[0] == n_var
        and ac.shape[0] == n_con
        and np.array_equal(av, np.arange(n_var, dtype=av.dtype))
        and np.array_equal(ac, np.arange(n_con, dtype=ac.dtype))
    )
    if not fast:
        return _reference_numpy(inputs)

    ncon_per = math.ceil(n_con / N_CORES)
    nvar_per = math.ceil((n_var - n_con) / N_CORES)

    key = (ncon_per, nvar_per)
    if key not in _NC_CACHE:
        _NC_CACHE[key] = _build_nc(ncon_per, nvar_per)
    nc = _NC_CACHE[key]

    in_maps = _make_in_maps(inputs, ncon_per, nvar_per)
    res = run_bass_kernel_spmd(nc, in_maps, core_ids=list(range(N_CORES)))

    out = np.empty(n_var, np.float32)
    oc = np.concatenate([r["out_con"] for r in res.results])
    ov = np.concatenate([r["out_var"] for r in res.results])
    out[:n_con] = oc[:n_con]
    out[n_con:] = ov[: n_var - n_con]
    return out


# revision 19
# speedup vs baseline: 1.0421x; 1.0421x over previous
"""Trainium2 Bass kernel for the GNN message-passing net (nn_Net_76690936037575).

Math: with assoc_var == arange(n_var) and assoc_con == arange(n_con) (the
spec-guaranteed fill), the scatter/scatter/gather pipeline collapses to

    out[0:n_con]      = head(con_mlp(con_node_features))
    out[n_con:n_var]  = head(var_mlp(var_node_features[n_con:n_var]))

where _mlp2 has no nonlinearity after its 2nd matmul, so that matmul fuses
with the head's 1st on the host:  M_s = sW2 @ W1, bM_s = sb2 @ W1 + b1.

Device dataflow (per core), activations laid out [128 dims x rows]:
  - 5 matmuls per 512-row tile in fp16 (K=2 input layer; 3x 128x128; the
    128->1 output layer col-tiled per PSUM bank via tile_position, with W4
    replicated to 32 cols so the whole bank is written).
  - bias+ReLU fused into ScalarE activation / VectorE tensor_scalar ops,
    one instruction per GROUP of 3 tiles over a 3-bank PSUM tile.
  - one Sigmoid per group; strided-partition DMA writes the output.

Sharding: rows split evenly across 8 cores (con 50k/core, var 25k/core),
weights replicated.
"""

import math

import numpy as np

DIM = 128
TILE_N = 512  # rows per matmul tile (one PSUM bank of f32)
GROUP = 4  # tiles per PSUM supertile / sigmoid group
N_CORES = 8

_NC_CACHE = {}


def _build_nc(ncon, nvar):
    """Build the per-core Bass program. ncon/nvar: rows per core."""
    import concourse.mybir as mybir
    import concourse.tile as tile
    from concourse import bacc

    dt = mybir.dt
    f32 = dt.float32
    f16 = dt.float16
    AF = mybir.ActivationFunctionType
    ALU = mybir.AluOpType

    nc = bacc.Bacc()

    GT = GROUP * TILE_N

    cfeat = nc.dram_tensor("cfeat", [2, ncon], f16, kind="ExternalInput")
    vfeat = nc.dram_tensor("vfeat", [2, nvar], f16, kind="ExternalInput")
    # all weights packed in one fp16 blob, all biases in one f32 blob, so the
    # constant preamble is 2 DMAs instead of 15 (keeps the warmup gap short)
    blob16 = nc.dram_tensor("blob16", [DIM, 800], f16, kind="ExternalInput")
    blob32 = nc.dram_tensor("blob32", [DIM, 7], f32, kind="ExternalInput")
    out_con = nc.dram_tensor("out_con", [ncon], f32, kind="ExternalOutput")
    out_var = nc.dram_tensor("out_var", [nvar], f32, kind="ExternalOutput")

    with tile.TileContext(nc) as tc:
        with (
            tc.tile_pool(name="const", bufs=1) as cpool,
            tc.tile_pool(name="feat", bufs=3) as fpool,
            tc.tile_pool(name="acts", bufs=10) as apool,
            tc.tile_pool(name="sig", bufs=2) as spool,
            tc.tile_pool(name="mm", bufs=2, space="PSUM") as mmpool,
        ):

            b16 = cpool.tile([DIM, 800], f16, tag="b16")
            nc.sync.dma_start(b16[:, :], blob16[:, :])
            b32 = cpool.tile([DIM, 7], f32, tag="b32")
            nc.sync.dma_start(b32[:, :], blob32[:, :])

            wmc_t = b16[:, 256:384]
            wmv_t = b16[:, 384:512]
            ww2_t = b16[:, 512:640]
            ww3_t = b16[:, 640:768]
            ww4_t = b16[:, 768:800]
            bb1c_t = b32[:, 0:1]
            bb1v_t = b32[:, 1:2]
            bbmc_t = b32[:, 2:3]
            bbmv_t = b32[:, 3:4]
            bb2_t = b32[:, 4:5]
            bb3_t = b32[:, 5:6]
            bb4_t = b32[:, 6:7]

            def ew_relu(t, p, bias, g_rows):
                """bias+ReLU over [128, g_rows]: front span on ScalarE,
                back span on VectorE, concurrently."""
                h = min((g_rows * 14) // 25, g_rows)
                if h > 0:
                    nc.scalar.activation(
                        t[:, 0:h], p[:, 0:h], AF.Relu, bias=bias[:, :]
                    )
                if h < g_rows:
                    nc.vector.tensor_scalar(
                        t[:, h:g_rows], p[:, h:g_rows], bias[:, :], 0.0,
                        ALU.add, ALU.max,
                    )

            def mm_layer(w_t, src_t, njs):
                """One 128-K layer over a group: len(njs) matmuls into one
                multi-bank PSUM tile."""
                p = mmpool.tile([DIM, GT], f32, tag="mm")
                for j, nj in enumerate(njs):
                    nc.tensor.matmul(
                        p[:, j * TILE_N : j * TILE_N + nj],
                        w_t[:, :],
                        src_t[:, j * TILE_N : j * TILE_N + nj],
                        start=True,
                        stop=True,
                    )
                return p

            def emit_group_layers(metas):
                """Emit 1-2 groups in lockstep: PE streams group B's matmuls
                while group A's ReLU drains, keeping PE dense."""
                fts, p1s, t1s, t2s, t3s, t4s, p5s = {}, {}, {}, {}, {}, {}, {}
                for gi, m in metas.items():
                    ft = fpool.tile([DIM, TILE_N], f16, tag="feat")
                    for j, nj in enumerate(m["njs"]):
                        off = m["g0"] + j * TILE_N
                        nc.sync.dma_start(
                            ft[32 * j : 32 * j + 2, :nj],
                            m["feat"][:, off : off + nj],
                        )
                    fts[gi] = ft
                # layer 1: h = relu(x @ A1 + a1), K=2, the group's tiles
                # row-packed into concurrent PE row-groups via tile_position
                for gi, m in metas.items():
                    p1 = mmpool.tile([DIM, GT], f32, tag="mm")
                    for j, nj in enumerate(m["njs"]):
                        nc.tensor.matmul(
                            p1[:, j * TILE_N : j * TILE_N + nj],
                            b16[32 * j : 32 * j + 2, m["wa1c0"] : m["wa1c0"] + DIM],
                            fts[gi][32 * j : 32 * j + 2, :nj],
                            start=True,
                            stop=True,
                            tile_position=(32 * j, 0),
                        )
                    p1s[gi] = p1
                for gi, m in metas.items():
                    t1 = apool.tile([DIM, GT], f16, tag="acts")
                    ew_relu(t1, p1s[gi], m["b1"], m["g_rows"])
                    t1s[gi] = t1
                # layer 2 (fused mlp2 + head1): u = relu(h @ M + bM)
                for gi, m in metas.items():
                    p2 = mm_layer(m["wm"], t1s[gi], m["njs"])
                    t2 = apool.tile([DIM, GT], f16, tag="acts")
                    ew_relu(t2, p2, m["bm"], m["g_rows"])
                    t2s[gi] = t2
                # layer 3: v = relu(u @ W2 + b2)
                for gi, m in metas.items():
                    p3 = mm_layer(ww2_t, t2s[gi], m["njs"])
                    t3 = apool.tile([DIM, GT], f16, tag="acts")
                    ew_relu(t3, p3, bb2_t, m["g_rows"])
                    t3s[gi] = t3
                # layer 4: w = relu(v @ W3 + b3)
                p4s = {}
                for gi, m in metas.items():
                    p4 = mm_layer(ww3_t, t3s[gi], m["njs"])
                    t4 = apool.tile([DIM, GT], f16, tag="acts")
                    ew_relu(t4, p4, bb3_t, m["g_rows"])
                    t4s[gi] = t4
                    p4s[gi] = p4
                # layer 5: y = sigmoid(w @ W4 + b4), col-tiled; output goes
                # into p4's first bank, which ReLU4 has just drained
                for gi, m in metas.items():
                    p5 = p4s[gi]
                    for j, nj in enumerate(m["njs"]):
                        nc.tensor.matmul(
                            p5[32 * j : 32 * j + 32, :nj],
                            ww4_t[:, :],
                            t4s[gi][:, j * TILE_N : j * TILE_N + nj],
                            start=True,
                            stop=True,
                            tile_position=(0, 32 * j),
                        )
                    p5s[gi] = p5
                for gi, m in metas.items():
                    p5 = p5s[gi]
                    sg = spool.tile([DIM, TILE_N], f32, tag="sig")
                    if m["full"]:
                        k = len(m["njs"])
                        pcov = 32 * k
                        pmax = 32 * (k - 1) + 1
                        nc.scalar.activation(
                            sg[:pcov, :], p5[:pcov, 0:TILE_N], AF.Sigmoid,
                            bias=bb4_t[:pcov, :],
                        )
                        nc.sync.dma_start(
                            m["out"][m["g0"] : m["g0"] + m["g_rows"]].rearrange(
                                "(a b) -> a b", b=TILE_N
                            ),
                            sg[0:pmax:32, :],
                        )
                    else:
                        for j, t in enumerate(m["tiles"]):
                            nj = m["njs"][j]
                            nc.scalar.activation(
                                sg[32 * j : 32 * j + 1, :nj],
                                p5[32 * j : 32 * j + 1, :nj],
                                AF.Sigmoid,
                                bias=bb4_t[32 * j : 32 * j + 1, :],
                            )
                            nc.sync.dma_start(
                                m["out"][t * TILE_N : t * TILE_N + nj].rearrange(
                                    "(a b) -> a b", a=1
                                ),
                                sg[32 * j : 32 * j + 1, :nj],
                            )

            def stream_groups(feat, n_rows, wa1c0, b1_t, wm_t, bm_t, out):
                n_tiles = math.ceil(n_rows / TILE_N)
                n_groups = math.ceil(n_tiles / GROUP)
                metas = []
                for gi in range(n_groups):
                    tiles = list(range(gi * GROUP, min((gi + 1) * GROUP, n_tiles)))
                    njs = [min(TILE_N, n_rows - t * TILE_N) for t in tiles]
                    metas.append({
                        "feat": feat,
                        "out": out,
                        "wa1c0": wa1c0,
                        "b1": b1_t,
                        "wm": wm_t,
                        "bm": bm_t,
                        "tiles": tiles,
                        "njs": njs,
                        "g0": gi * GT,
                        "g_rows": min(GT, n_rows - gi * GT),
                        "full": all(n == TILE_N for n in njs),
                    })
                return metas

            # HAM warmup: ~5us of dense back-to-back matmuls flips the PE
            # clock gate from 1.2 to 2.4 GHz before the main stream begins.
            wp = mmpool.tile([DIM, GT], f32, tag="mm")
            for _ in range(14):
                nc.tensor.matmul(
                    wp[:, :TILE_N], b16[:, 0:128], b16[:, 288:800],
                    start=True, stop=True,
                )

            con_groups = stream_groups(
                cfeat, ncon, 0, bb1c_t, wmc_t, bbmc_t, out_con
            )
            var_groups = stream_groups(
                vfeat, nvar, 128, bb1v_t, wmv_t, bbmv_t, out_var
            )
            # interleave the two independent streams so one group's matmuls
            # overlap the other's ReLU drains
            order = []
            i = j = 0
            while i < len(con_groups) or j < len(var_groups):
                if i < len(con_groups):
                    order.append(con_groups[i]); i += 1
                if j < len(var_groups):
                    order.append(var_groups[j]); j += 1
            k = 0
            while k < len(order):
                pair = order[k : k + 2]
                emit_group_layers({n: m for n, m in enumerate(pair)})
                k += len(pair)

    nc.compile()
    return nc


def _make_in_maps(inputs, ncon_per, nvar_per):
    """Host-side sharding: transpose features, split rows, fuse weights."""
    f32 = np.float32
    f16 = np.float16
    cf = np.asarray(inputs["con_node_features"], f32)
    vf = np.asarray(inputs["var_node_features"], f32)
    n_con = cf.shape[0]
    n_var = vf.shape[0]

    W1 = np.asarray(inputs["W1"], f32)
    b1 = np.asarray(inputs["b1"], f32)
    mc = np.asarray(inputs["cW2"], f32) @ W1
    bmc = np.asarray(inputs["cb2"], f32) @ W1 + b1
    mv = np.asarray(inputs["vW2"], f32) @ W1
    bmv = np.asarray(inputs["vb2"], f32) @ W1 + b1

    conT = np.zeros((2, ncon_per * N_CORES), f16)
    conT[:, :n_con] = cf.T
    varT = np.zeros((2, nvar_per * N_CORES), f16)
    varT[:, : n_var - n_con] = vf[n_con:].T

    blob16 = np.zeros((DIM, 800), f16)
    for j in range(4):
        blob16[32 * j : 32 * j + 2, 0:128] = np.asarray(
            inputs["cW1"], f32
        ).astype(f16)
        blob16[32 * j : 32 * j + 2, 128:256] = np.asarray(
            inputs["vW1"], f32
        ).astype(f16)
    blob16[:, 256:384] = mc.astype(f16)
    blob16[:, 384:512] = mv.astype(f16)
    blob16[:, 512:640] = np.asarray(inputs["W2"], f32).astype(f16)
    blob16[:, 640:768] = np.asarray(inputs["W3"], f32).astype(f16)
    blob16[:, 768:800] = np.repeat(
        np.asarray(inputs["W4"], f32).reshape(DIM, 1), 32, axis=1
    ).astype(f16)
    blob32 = np.zeros((DIM, 7), f32)
    blob32[:, 0] = np.asarray(inputs["cb1"], f32)
    blob32[:, 1] = np.asarray(inputs["vb1"], f32)
    blob32[:, 2] = bmc
    blob32[:, 3] = bmv
    blob32[:, 4] = np.asarray(inputs["b2"], f32)
    blob32[:, 5] = np.asarray(inputs["b3"], f32)
    blob32[:, 6] = np.asarray(inputs["b4"], f32).reshape(-1)[0]

    shared = {"blob16": blob16, "blob32": blob32}
    in_maps = []
    for i in range(N_CORES):
        m = dict(shared)
        m["cfeat"] = np.ascontiguousarray(conT[:, i * ncon_per : (i + 1) * ncon_per])
        m["vfeat"] = np.ascontiguousarray(varT[:, i * nvar_per : (i + 1) * nvar_per])
        in_maps.append(m)
    return in_maps


def _reference_numpy(inputs):
    """General fallback (non-arange assoc indices): plain numpy."""
    f32 = np.float32

    def mlp2(x, W1, b1, W2, b2):
        return np.maximum(x @ W1 + b1, 0.0) @ W2 + b2

    vf = np.asarray(inputs["var_node_features"], f32)
    cf = np.asarray(inputs["con_node_features"], f32)
    av = np.asarray(inputs["assoc_var"])
    ac = np.asarray(inputs["assoc_con"])
    n = mlp2(vf, inputs["vW1"], inputs["vb1"], inputs["vW2"], inputs["vb2"])
    e = mlp2(cf, inputs["cW1"], inputs["cb1"], inputs["cW2"], inputs["cb2"])
    x = np.zeros((np.asarray(inputs["node_types"]).shape[0], n.shape[-1]), f32)
    x[av] = n
    x[ac] = e
    x = x[av]
    x = np.maximum(x @ inputs["W1"] + inputs["b1"], 0.0)
    x = np.maximum(x @ inputs["W2"] + inputs["b2"], 0.0)
    x = np.maximum(x @ inputs["W3"] + inputs["b3"], 0.0)
    x = x @ inputs["W4"] + inputs["b4"]
    return (1.0 / (1.0 + np.exp(-x))).astype(f32).squeeze(-1)


def kernel(**inputs):
    from concourse.bass_utils import run_bass_kernel_spmd

    cf = np.asarray(inputs["con_node_features"])
    vf = np.asarray(inputs["var_node_features"])
    av = np.asarray(inputs["assoc_var"])
    ac = np.asarray(inputs["assoc_con"])
    n_con = cf.shape[0]
    n_var = vf.shape[0]

    fast = (
        n_con <= n_var
        and av.shape[0] == n_var
        and ac.shape[0] == n_con
        and np.array_equal(av, np.arange(n_var, dtype=av.dtype))
        and np.array_equal(ac, np.arange(n_con, dtype=ac.dtype))
    )
    if not fast:
        return _reference_numpy(inputs)

    ncon_per = math.ceil(n_con / N_CORES)
    nvar_per = math.ceil((n_var - n_con) / N_CORES)

    key = (ncon_per, nvar_per)
    if key not in _NC_CACHE:
        _NC_CACHE[key] = _build_nc(ncon_per, nvar_per)
    nc = _NC_CACHE[key]

    in_maps = _make_in_maps(inputs, ncon_per, nvar_per)
    res = run_bass_kernel_spmd(nc, in_maps, core_ids=list(range(N_CORES)))

    out = np.empty(n_var, np.float32)
    oc = np.concatenate([r["out_con"] for r in res.results])
    ov = np.concatenate([r["out_var"] for r in res.results])
    out[:n_con] = oc[:n_con]
    out[n_con:] = ov[: n_var - n_con]
    return out
